# revision 12
# baseline (speedup 1.0000x reference)
"""Causal self-attention (QK-RMSNorm + RoPE) Trainium2 kernel.

Sharding: 8 cores = 4 batches x 2 head-groups (Megatron-style over heads).
Core c handles batch b=c//2, heads [g*8, g*8+8) with g=c%2.
Each core computes y[b, :, g*512:(g+1)*512] (output-column sharding of the
projection after a pairwise AllGather of attention outputs), so the host
only concatenates slices - no host-side arithmetic.

PE-cost notes (cost model charges out-free-size rows per matmul, independent
of contraction depth and output-partition count):
- V tiles carry 64 extra all-ones columns, so each AV matmul emits the
  attention numerator on PSUM partitions 0:64 AND the softmax denominator
  replicated on partitions 64:128 - no separate ones@pt matmuls.
- The per-head sum-of-squares for QK-RMSNorm uses one block-diagonal-ones
  matmul covering both packed heads instead of two half-array matmuls.
"""


import numpy as np
import ml_dtypes

import concourse.bass as bass
import concourse.bacc as bacc

# Force all activations into the one table set that covers Exp+Ln+Square+
# Copy+Identity, so the whole kernel needs exactly one ACT_TABLE_LOAD.
import concourse.hw_specs as _hw_specs
_orig_gat = _hw_specs.get_activation_tables

def _gat_one_set(arch):
    t = _orig_gat(arch)
    return {k: (v if k == "natural_log_exp_and_others" else set())
            for k, v in t.items()}

bacc.get_activation_tables = _gat_one_set
import concourse.mybir as mybir
import concourse.tile as tile
from concourse.bass_utils import run_bass_kernel_spmd

BF16 = mybir.dt.bfloat16
F32 = mybir.dt.float32

N_HEAD = 16
HEAD_DIM = 64
EPS = 1e-5
ROPE_BASE = 10000.0

B, T, C = 4, 2048, 1024
H_LOCAL = N_HEAD // 2          # heads per core
PAIRS = H_LOCAL // 2           # head-pairs per core (processed 2-at-a-time)
CT = C // 128                  # contraction tiles over C
FL = H_LOCAL * HEAD_DIM        # local feature width (512)
QCH = 512                      # q-chunk width
NQC = T // QCH                 # q-chunks
NKT = T // 128                 # k tiles
NTT = T // 128                 # token tiles

_cached = {}


def _fbcast2(ap):
    """[128, N] AP -> [128, 2, N] with the middle (free) dim broadcast."""
    return bass.AP(
        tensor=ap.tensor, offset=ap.offset, ap=[ap.ap[0], [0, 2], ap.ap[1]]
    )


def _rope_tables():
    inv_freq = 1.0 / (ROPE_BASE ** (np.arange(0, HEAD_DIM, 2, dtype=np.float64) / HEAD_DIM))
    t = np.arange(T, dtype=np.float64)
    freqs = np.outer(t, inv_freq)                       # [T, 32]
    emb = np.concatenate([freqs, freqs], -1)            # [T, 64]
    cos = np.cos(emb).astype(np.float32).T              # [64, T]
    sin = np.sin(emb).astype(np.float32).T              # [64, T]
    cos2 = np.concatenate([cos, cos], 0)                # [128, T] two heads
    sin_s = sin.copy()
    sin_s[0:32] = -sin_s[0:32]                          # rotate-half sign
    sin2 = np.concatenate([sin_s, sin_s], 0)            # [128, T]
    return cos2.astype(ml_dtypes.bfloat16), sin2.astype(ml_dtypes.bfloat16)


def _diag_masks():
    # corner mask: keep where k_partition <= q_col (lower-triangular 128x128)
    p = np.arange(128)[:, None]
    qf = np.arange(128)[None, :]
    m = (p <= qf).astype(np.float32)
    return m.astype(ml_dtypes.bfloat16)                 # [128, 128]


def build_program(no_cc=False):
    nc = bacc.Bacc("TRN2", target_bir_lowering=False, debug=False,
                   num_devices=1 if no_cc else 8)

    xT_d = nc.dram_tensor("xT", [C, T], BF16, kind="ExternalInput")
    wq_d = nc.dram_tensor("Wq", [C, FL], BF16, kind="ExternalInput")
    wk_d = nc.dram_tensor("Wk", [C, FL], BF16, kind="ExternalInput")
    wv_d = nc.dram_tensor("Wv", [C, FL], BF16, kind="ExternalInput")
    wp_d = nc.dram_tensor("Wp", [C, FL], BF16, kind="ExternalInput")
    y_d = nc.dram_tensor("y", [T, FL], F32, kind="ExternalOutput")

    cos2_np, sin2_np = _rope_tables()
    cos_d = nc.inline_tensor(np.ascontiguousarray(cos2_np), "cos2")
    sin_d = nc.inline_tensor(np.ascontiguousarray(sin2_np), "sin2")
    mask_d = nc.inline_tensor(np.ascontiguousarray(_diag_masks()), "masks")

    # per-pair exchange buffers
    cc_ins = [nc.dram_tensor(f"cc_in{p}", [128, T], BF16) for p in range(PAIRS)]
    cc_outs = [nc.dram_tensor(f"cc_out{p}", [2, 128, T], BF16) for p in range(PAIRS)]

    from contextlib import ExitStack
    with tile.TileContext(nc) as tc:
        with (
            tc.tile_pool(name="const", bufs=1) as const,
            tc.tile_pool(name="stats", bufs=8) as work,
            tc.tile_pool(name="evw", bufs=4) as evw,
            tc.tile_pool(name="rope", bufs=4) as ropep,
            tc.tile_pool(name="pt", bufs=5) as ptp,
            tc.tile_pool(name="ps_s2", bufs=2, space="PSUM") as ps_s2,
            tc.tile_pool(name="ps_b", bufs=4, space="PSUM") as ps_b,
        ):
            early_ctx = ExitStack()
            early = early_ctx.enter_context(tc.tile_pool(name="early", bufs=1))

            # ---- constants / inputs (wv + xT first: v-proj consumes them) ----
            wv_sb = early.tile([128, CT, FL], BF16)
            xT_sb = early.tile([128, CT, T], BF16)
            for k in range(CT):
                nc.sync.dma_start(wv_sb[:, k, :], wv_d[k * 128:(k + 1) * 128, :])
                nc.sync.dma_start(xT_sb[:, k, :], xT_d[k * 128:(k + 1) * 128, :])
            wq_sb = early.tile([128, CT, FL], BF16)
            nc.sync.dma_start(wq_sb[:], wq_d[:].rearrange("(k p) f -> p k f", p=128))
            wk_sb = early.tile([128, CT, FL], BF16)
            nc.sync.dma_start(wk_sb[:], wk_d[:].rearrange("(k p) f -> p k f", p=128))
            cos_sb = early.tile([128, T], BF16)
            nc.sync.dma_start(cos_sb[:], cos_d[:])
            sin_sb = early.tile([128, T], BF16)
            nc.sync.dma_start(sin_sb[:], sin_d[:])
            mask_sb = early.tile([128, 128], BF16)
            nc.sync.dma_start(mask_sb[:], mask_d[:])
            wp_sb = const.tile([128, CT, FL], BF16)
            nc.sync.dma_start(wp_sb[:], wp_d[:].rearrange("(k p) f -> p k f", p=128))

            # block-diagonal ones: one matmul sums squares of both packed heads
            ones_bd = const.tile([128, 128], BF16)
            nc.gpsimd.memset(ones_bd[:], 0.0)
            nc.gpsimd.memset(ones_bd[0:64, 0:64], 1.0)
            nc.gpsimd.memset(ones_bd[64:128, 64:128], 1.0)

            qk_sb = const.tile([128, 2 * PAIRS, T], BF16)
            # V with 64 all-ones columns per head: AV matmul then yields the
            # softmax denominator broadcast on PSUM partitions 64:128.
            v_sb = const.tile([128, NTT, H_LOCAL, 128], BF16)
            nc.gpsimd.memset(v_sb[:, :, :, 64:128], 1.0)
            aoT_sb = const.tile([128, PAIRS, T], BF16)

            def qkv_mtile(m):
                w_sb = wq_sb if m < PAIRS else wk_sb
                mloc = (m % PAIRS) * 128

                def norm_tail(pss, sq, n):
                    # PE's ss matmul runs one chunk behind the mains so it
                    # never blocks them waiting on the square (in-order PE).
                    ss = ps_b.tile([128, QCH], F32, tag="mm")
                    nc.tensor.matmul(ss, lhsT=ones_bd[:], rhs=sq[:],
                                     start=True, stop=True,
                                     skip_group_check=True)
                    # rstd = (ss/64)^(-1/2) = exp(-0.5*ln(ss/64)); eps is
                    # negligible vs mean-square ~1. ln+exp live in one ACT
                    # table set (natural_log_exp_and_others) -> no set thrash.
                    rr = work.tile([128, QCH], F32, tag="st")
                    nc.scalar.activation(rr[:], ss[:],
                                         mybir.ActivationFunctionType.Ln,
                                         scale=1.0 / HEAD_DIM)
                    rstd = work.tile([128, QCH], F32, tag="st")
                    nc.scalar.activation(rstd[:], rr[:],
                                         mybir.ActivationFunctionType.Exp,
                                         scale=-0.5)
                    dst = qk_sb[:, m, n * QCH:(n + 1) * QCH]
                    nc.vector.tensor_mul(dst, pss, rstd[:])

                pend = None
                for n in range(T // QCH):
                    pss = ps_b.tile([128, QCH], F32, tag="mm")
                    for k in range(CT):
                        nc.tensor.matmul(
                            pss,
                            lhsT=w_sb[:, k, mloc:mloc + 128],
                            rhs=xT_sb[:, k, n * QCH:(n + 1) * QCH],
                            start=(k == 0),
                            stop=(k == CT - 1),
                            skip_group_check=True,
                        )
                    # square must be Act: DVE cannot read two PSUM operands
                    sq = work.tile([128, QCH], BF16, tag="st")
                    nc.scalar.activation(sq[:], pss,
                                         mybir.ActivationFunctionType.Square)
                    if pend is not None:
                        norm_tail(*pend)
                    pend = (pss, sq, n)
                norm_tail(*pend)

            def rope_mtile(m):
                src = qk_sb[:, m, :]
                # rotate-half copies on GPSIMD (SBUF-only op, Pool is idle)
                sw = ropep.tile([128, T], BF16, tag="rp")
                for off in (0, 64):
                    nc.gpsimd.tensor_copy(sw[off:off + 32, :], src[off + 32:off + 64, :])
                    nc.gpsimd.tensor_copy(sw[off + 32:off + 64, :], src[off:off + 32, :])
                t1 = ropep.tile([128, T], BF16, tag="rp")
                nc.vector.tensor_mul(t1[:], src, cos_sb[:])
                nc.vector.tensor_mul(sw[:], sw[:], sin_sb[:])
                nc.vector.tensor_add(src, t1[:], sw[:])

            def attention_pair(p):
                qT = qk_sb[:, p, :]
                kT = qk_sb[:, PAIRS + p, :]
                hA, hB = 2 * p, 2 * p + 1
                for cqi in range(NQC):
                    kmax = (cqi + 1) * (QCH // 128)
                    ypsA = ps_b.tile([128, QCH], F32, tag="mm")
                    ypsB = ps_b.tile([128, QCH], F32, tag="mm")

                    def issue_av(pt, j, q0):
                        st, sp = (j == 0), (j == kmax - 1)
                        nc.tensor.matmul(ypsA[:, q0:QCH],
                                         lhsT=v_sb[:, j, hA, :],
                                         rhs=pt[:, 0, q0:QCH], start=st, stop=sp,
                                         skip_group_check=True)
                        nc.tensor.matmul(ypsB[:, q0:QCH],
                                         lhsT=v_sb[:, j, hB, :],
                                         rhs=pt[:, 1, q0:QCH], start=st, stop=sp,
                                         skip_group_check=True)

                    pend_av = None
                    for j in range(kmax):
                        jr = j - cqi * (QCH // 128)
                        q0 = max(jr, 0) * 128
                        sq_sl = slice(cqi * QCH + q0, (cqi + 1) * QCH)
                        s2 = ps_s2.tile([128, 2, QCH], F32, tag="s2")
                        nc.tensor.matmul(s2[:, 0, q0:QCH],
                                         lhsT=kT[0:64, j * 128:(j + 1) * 128],
                                         rhs=qT[0:64, sq_sl], start=True, stop=True,
                                         skip_group_check=True)
                        nc.tensor.matmul(s2[:, 1, q0:QCH],
                                         lhsT=kT[64:128, j * 128:(j + 1) * 128],
                                         rhs=qT[64:128, sq_sl], start=True, stop=True,
                                         skip_group_check=True)
                        pt = ptp.tile([128, 2, QCH], BF16, tag="pt")
                        nc.scalar.activation(pt[:, :, q0:QCH], s2[:, :, q0:QCH],
                                             mybir.ActivationFunctionType.Exp,
                                             scale=0.125)
                        if jr >= 0:
                            ptc = pt[:, :, q0:q0 + 128]
                            nc.vector.tensor_mul(ptc, ptc, _fbcast2(mask_sb[:]))
                        # AV(j) is issued after QK(j+1) so the in-order PE
                        # never parks on exp(j); exp runs under QK(j+1).
                        if pend_av is not None:
                            issue_av(*pend_av)
                        pend_av = (pt, j, q0)
                    issue_av(*pend_av)
                    sl = slice(cqi * QCH, (cqi + 1) * QCH)
                    drA = evw.tile([64, QCH], F32, tag="ev")
                    nc.vector.reciprocal_approx_fast(drA[:], ypsA[64:128, :])
                    drB = evw.tile([64, QCH], F32, tag="ev")
                    nc.vector.reciprocal_approx_fast(drB[:], ypsB[64:128, :])
                    # (GPSIMD cannot read PSUM on real hw - keep these on DVE)
                    nc.vector.tensor_mul(aoT_sb[0:64, p, sl], ypsA[0:64, :], drA[:])
                    nc.vector.tensor_mul(aoT_sb[64:128, p, sl], ypsB[0:64, :], drB[:])

            # ---- v projection first (needed by every attention pair) ----
            for tt in range(NTT):
                pss = ps_b.tile([128, FL], F32, tag="mm")
                for k in range(CT):
                    nc.tensor.matmul(
                        pss,
                        lhsT=xT_sb[:, k, tt * 128:(tt + 1) * 128],
                        rhs=wv_sb[:, k, :],
                        start=(k == 0),
                        stop=(k == CT - 1),
                    )
                nc.scalar.copy(v_sb[:, tt, :, 0:64], pss)

            # ---- per-pair: qkv -> rope -> attention -> exchange ----
            # qkv+rope of pair p+1 are issued before attention(p) so rope's
            # DVE work overlaps attention's PE work instead of stalling it.
            qkv_mtile(0)
            qkv_mtile(PAIRS)
            rope_mtile(0)
            rope_mtile(PAIRS)
            for p in range(PAIRS):
                if p + 1 < PAIRS:
                    qkv_mtile(p + 1)
                    qkv_mtile(PAIRS + p + 1)
                    rope_mtile(p + 1)
                    rope_mtile(PAIRS + p + 1)
                attention_pair(p)
                nc.sync.dma_start(cc_ins[p][:], aoT_sb[:, p, :])
                if not no_cc:
                    nc.gpsimd.collective_compute(
                        "AllGather",
                        mybir.AluOpType.bypass,
                        replica_groups=[[0, 1], [2, 3], [4, 5], [6, 7]],
                        ins=[cc_ins[p][:].opt()],
                        outs=[cc_outs[p][:].opt()],
                    )
                # qk slots p and 4+p are dead after attention p: receive the
                # gathered pair there (slot index == global f-tile index)
                if no_cc:
                    nc.sync.dma_start(qk_sb[:, p, :], cc_ins[p][:])
                    nc.sync.dma_start(qk_sb[:, PAIRS + p, :], cc_ins[p][:])
                else:
                    nc.sync.dma_start(qk_sb[:, p, :], cc_outs[p][0])
                    nc.sync.dma_start(qk_sb[:, PAIRS + p, :], cc_outs[p][1])

            early_ctx.close()

            # ---- projection over all 8 global f-tiles (rank-independent) ----
            for tt in range(NTT):
                pss = ps_b.tile([128, FL], F32, tag="mm")
                # kf in exchange-arrival order (pair p delivers kf=p and kf=4+p)
                kf_order = [kf for p in range(PAIRS) for kf in (p, PAIRS + p)]
                for i, kf in enumerate(kf_order):
                    nc.tensor.matmul(
                        pss,
                        lhsT=qk_sb[:, kf, tt * 128:(tt + 1) * 128],
                        rhs=wp_sb[:, kf, :],
                        start=(i == 0),
                        stop=(i == 2 * PAIRS - 1),
                    )
                ysb = evw.tile([128, FL], F32, tag="ev")
                nc.scalar.copy(ysb[:], pss)
                nc.sync.dma_start(y_d[tt * 128:(tt + 1) * 128, :], ysb[:])

    nc.compile()
    return nc


def _prep_core_inputs(x, Wqkv, Wproj, q_norm_w, k_norm_w, core):
    b, g = core // 2, core % 2
    bf = ml_dtypes.bfloat16
    xT = np.ascontiguousarray(x[b].T).astype(bf)
    cols = slice(g * FL, (g + 1) * FL)
    wq = Wqkv[:, 0:C][:, cols] * np.tile(q_norm_w, H_LOCAL)[None, :]
    wk = Wqkv[:, C:2 * C][:, cols] * np.tile(k_norm_w, H_LOCAL)[None, :]
    wv = Wqkv[:, 2 * C:3 * C][:, cols]
    wp = Wproj[:, cols]
    return {
        "xT": xT,
        "Wq": np.ascontiguousarray(wq).astype(bf),
        "Wk": np.ascontiguousarray(wk).astype(bf),
        "Wv": np.ascontiguousarray(wv).astype(bf),
        "Wp": np.ascontiguousarray(wp).astype(bf),
    }


def kernel(x, Wqkv, Wproj, q_norm_w, k_norm_w):
    if "nc" not in _cached:
        _cached["nc"] = build_program()
    nc = _cached["nc"]

    x = np.asarray(x, dtype=np.float32)
    Wqkv = np.asarray(Wqkv, dtype=np.float32)
    Wproj = np.asarray(Wproj, dtype=np.float32)
    q_norm_w = np.asarray(q_norm_w, dtype=np.float32)
    k_norm_w = np.asarray(k_norm_w, dtype=np.float32)

    in_maps = [
        _prep_core_inputs(x, Wqkv, Wproj, q_norm_w, k_norm_w, c) for c in range(8)
    ]
    res = run_bass_kernel_spmd(nc, in_maps, list(range(8)))
    outs = res.results

    y = np.empty((B, T, C), dtype=np.float32)
    for b in range(B):
        y[b, :, 0:FL] = outs[2 * b]["y"]
        y[b, :, FL:C] = outs[2 * b + 1]["y"]
    return y


# revision 13
# speedup vs baseline: 1.0868x; 1.0868x over previous
"""Causal self-attention (QK-RMSNorm + RoPE) Trainium2 kernel.

Sharding: 8 cores = 4 batches x 2 head-groups (Megatron-style over heads).
Core c handles batch b=c//2, heads [g*8, g*8+8) with g=c%2.
Each core computes y[b, :, g*512:(g+1)*512] (output-column sharding of the
projection after a pairwise AllGather of attention outputs), so the host
only concatenates slices - no host-side arithmetic.

PE-cost notes (cost model charges out-free-size rows per matmul, independent
of contraction depth and output-partition count):
- V tiles carry 64 extra all-ones columns, so each AV matmul emits the
  attention numerator on PSUM partitions 0:64 AND the softmax denominator
  replicated on partitions 64:128 - no separate ones@pt matmuls.
- The per-head sum-of-squares for QK-RMSNorm uses one block-diagonal-ones
  matmul covering both packed heads instead of two half-array matmuls.
"""


import numpy as np
import ml_dtypes

import concourse.bass as bass
import concourse.bacc as bacc

# Force all activations into the one table set that covers Exp+Ln+Square+
# Copy+Identity, so the whole kernel needs exactly one ACT_TABLE_LOAD.
import concourse.hw_specs as _hw_specs
_orig_gat = _hw_specs.get_activation_tables

def _gat_one_set(arch):
    t = _orig_gat(arch)
    return {k: (v if k == "natural_log_exp_and_others" else set())
            for k, v in t.items()}

bacc.get_activation_tables = _gat_one_set
import concourse.mybir as mybir
import concourse.tile as tile
from concourse.bass_utils import run_bass_kernel_spmd

BF16 = mybir.dt.bfloat16
F32 = mybir.dt.float32

N_HEAD = 16
HEAD_DIM = 64
EPS = 1e-5
ROPE_BASE = 10000.0

B, T, C = 4, 2048, 1024
H_LOCAL = N_HEAD // 2          # heads per core
PAIRS = H_LOCAL // 2           # head-pairs per core (processed 2-at-a-time)
CT = C // 128                  # contraction tiles over C
FL = H_LOCAL * HEAD_DIM        # local feature width (512)
QCH = 512                      # q-chunk width
NQC = T // QCH                 # q-chunks
NKT = T // 128                 # k tiles
NTT = T // 128                 # token tiles

_cached = {}


def _fbcast2(ap):
    """[128, N] AP -> [128, 2, N] with the middle (free) dim broadcast."""
    return bass.AP(
        tensor=ap.tensor, offset=ap.offset, ap=[ap.ap[0], [0, 2], ap.ap[1]]
    )


def _rope_tables():
    inv_freq = 1.0 / (ROPE_BASE ** (np.arange(0, HEAD_DIM, 2, dtype=np.float64) / HEAD_DIM))
    t = np.arange(T, dtype=np.float64)
    freqs = np.outer(t, inv_freq)                       # [T, 32]
    emb = np.concatenate([freqs, freqs], -1)            # [T, 64]
    cos = np.cos(emb).astype(np.float32).T              # [64, T]
    sin = np.sin(emb).astype(np.float32).T              # [64, T]
    cos2 = np.concatenate([cos, cos], 0)                # [128, T] two heads
    sin_s = sin.copy()
    sin_s[0:32] = -sin_s[0:32]                          # rotate-half sign
    sin2 = np.concatenate([sin_s, sin_s], 0)            # [128, T]
    return cos2.astype(ml_dtypes.bfloat16), sin2.astype(ml_dtypes.bfloat16)


def _diag_masks():
    # corner mask: keep where k_partition <= q_col (lower-triangular 128x128)
    p = np.arange(128)[:, None]
    qf = np.arange(128)[None, :]
    m = (p <= qf).astype(np.float32)
    return m.astype(ml_dtypes.bfloat16)                 # [128, 128]


def build_program(no_cc=False):
    nc = bacc.Bacc("TRN2", target_bir_lowering=False, debug=False,
                   num_devices=1 if no_cc else 8)

    xT_d = nc.dram_tensor("xT", [C, T], BF16, kind="ExternalInput")
    wq_d = nc.dram_tensor("Wq", [C, FL], BF16, kind="ExternalInput")
    wk_d = nc.dram_tensor("Wk", [C, FL], BF16, kind="ExternalInput")
    wv_d = nc.dram_tensor("Wv", [C, FL], BF16, kind="ExternalInput")
    wp_d = nc.dram_tensor("Wp", [C, FL], BF16, kind="ExternalInput")
    y_d = nc.dram_tensor("y", [T, FL], F32, kind="ExternalOutput")

    cos2_np, sin2_np = _rope_tables()
    cos_d = nc.inline_tensor(np.ascontiguousarray(cos2_np), "cos2")
    sin_d = nc.inline_tensor(np.ascontiguousarray(sin2_np), "sin2")
    mask_d = nc.inline_tensor(np.ascontiguousarray(_diag_masks()), "masks")

    # per-pair exchange buffers
    cc_ins = [nc.dram_tensor(f"cc_in{p}", [128, T], BF16) for p in range(PAIRS)]
    cc_outs = [nc.dram_tensor(f"cc_out{p}", [2, 128, T], BF16) for p in range(PAIRS)]

    from contextlib import ExitStack
    with tile.TileContext(nc) as tc:
        with (
            tc.tile_pool(name="const", bufs=1) as const,
            tc.tile_pool(name="stats", bufs=8) as work,
            tc.tile_pool(name="evw", bufs=4) as evw,
            tc.tile_pool(name="rope", bufs=4) as ropep,
            tc.tile_pool(name="pt", bufs=5) as ptp,
            tc.tile_pool(name="ps_s2", bufs=2, space="PSUM") as ps_s2,
            tc.tile_pool(name="ps_b", bufs=4, space="PSUM") as ps_b,
        ):
            early_ctx = ExitStack()
            early = early_ctx.enter_context(tc.tile_pool(name="early", bufs=1))

            # ---- constants / inputs (wv + xT first: v-proj consumes them) ----
            wv_sb = early.tile([128, CT, FL], BF16)
            xT_sb = early.tile([128, CT, T], BF16)
            for k in range(CT):
                nc.sync.dma_start(wv_sb[:, k, :], wv_d[k * 128:(k + 1) * 128, :])
                nc.sync.dma_start(xT_sb[:, k, :], xT_d[k * 128:(k + 1) * 128, :])
            wq_sb = early.tile([128, CT, FL], BF16)
            nc.sync.dma_start(wq_sb[:], wq_d[:].rearrange("(k p) f -> p k f", p=128))
            wk_sb = early.tile([128, CT, FL], BF16)
            nc.sync.dma_start(wk_sb[:], wk_d[:].rearrange("(k p) f -> p k f", p=128))
            cos_sb = early.tile([128, T], BF16)
            nc.sync.dma_start(cos_sb[:], cos_d[:])
            sin_sb = early.tile([128, T], BF16)
            nc.sync.dma_start(sin_sb[:], sin_d[:])
            mask_sb = early.tile([128, 128], BF16)
            nc.sync.dma_start(mask_sb[:], mask_d[:])
            wp_sb = const.tile([128, CT, FL], BF16)
            nc.sync.dma_start(wp_sb[:], wp_d[:].rearrange("(k p) f -> p k f", p=128))

            # block-diagonal ones: one matmul sums squares of both packed heads
            ones_bd = const.tile([128, 128], BF16)
            nc.gpsimd.memset(ones_bd[:], 0.0)
            nc.gpsimd.memset(ones_bd[0:64, 0:64], 1.0)
            nc.gpsimd.memset(ones_bd[64:128, 64:128], 1.0)

            qk_sb = const.tile([128, 2 * PAIRS, T], BF16)
            # V with 64 all-ones columns per head: AV matmul then yields the
            # softmax denominator broadcast on PSUM partitions 64:128.
            v_sb = const.tile([128, NTT, H_LOCAL, 128], BF16)
            nc.gpsimd.memset(v_sb[:, :, :, 64:128], 1.0)
            aoT_sb = const.tile([128, PAIRS, T], BF16)

            def qkv_mtile(m):
                w_sb = wq_sb if m < PAIRS else wk_sb
                mloc = (m % PAIRS) * 128

                def norm_tail(pss, sq, n):
                    # PE's ss matmul runs one chunk behind the mains so it
                    # never blocks them waiting on the square (in-order PE).
                    ss = ps_b.tile([128, QCH], F32, tag="mm")
                    nc.tensor.matmul(ss, lhsT=ones_bd[:], rhs=sq[:],
                                     start=True, stop=True,
                                     skip_group_check=True)
                    # rstd = (ss/64)^(-1/2) = exp(-0.5*ln(ss/64)); eps is
                    # negligible vs mean-square ~1. ln+exp live in one ACT
                    # table set (natural_log_exp_and_others) -> no set thrash.
                    rr = work.tile([128, QCH], F32, tag="st")
                    nc.scalar.activation(rr[:], ss[:],
                                         mybir.ActivationFunctionType.Ln,
                                         scale=1.0 / HEAD_DIM)
                    rstd = work.tile([128, QCH], F32, tag="st")
                    nc.scalar.activation(rstd[:], rr[:],
                                         mybir.ActivationFunctionType.Exp,
                                         scale=-0.5)
                    dst = qk_sb[:, m, n * QCH:(n + 1) * QCH]
                    nc.vector.tensor_mul(dst, pss, rstd[:])

                pend = None
                for n in range(T // QCH):
                    pss = ps_b.tile([128, QCH], F32, tag="mm")
                    for k in range(CT):
                        nc.tensor.matmul(
                            pss,
                            lhsT=w_sb[:, k, mloc:mloc + 128],
                            rhs=xT_sb[:, k, n * QCH:(n + 1) * QCH],
                            start=(k == 0),
                            stop=(k == CT - 1),
                            skip_group_check=True,
                        )
                    # square must be Act: DVE cannot read two PSUM operands
                    sq = work.tile([128, QCH], BF16, tag="st")
                    nc.scalar.activation(sq[:], pss,
                                         mybir.ActivationFunctionType.Square)
                    if pend is not None:
                        norm_tail(*pend)
                    pend = (pss, sq, n)
                norm_tail(*pend)

            def rope_mtile(m):
                src = qk_sb[:, m, :]
                sw = ropep.tile([128, T], BF16, tag="rp")
                for off in (0, 64):
                    nc.vector.tensor_copy(sw[off:off + 32, :], src[off + 32:off + 64, :])
                    nc.vector.tensor_copy(sw[off + 32:off + 64, :], src[off:off + 32, :])
                t1 = ropep.tile([128, T], BF16, tag="rp")
                nc.vector.tensor_mul(t1[:], src, cos_sb[:])
                nc.vector.tensor_mul(sw[:], sw[:], sin_sb[:])
                nc.vector.tensor_add(src, t1[:], sw[:])

            def attention_pair(p):
                qT = qk_sb[:, p, :]
                kT = qk_sb[:, PAIRS + p, :]
                hA, hB = 2 * p, 2 * p + 1
                for cqi in range(NQC):
                    kmax = (cqi + 1) * (QCH // 128)
                    ypsA = ps_b.tile([128, QCH], F32, tag="mm")
                    ypsB = ps_b.tile([128, QCH], F32, tag="mm")

                    def issue_av(pt, j, q0):
                        st, sp = (j == 0), (j == kmax - 1)
                        nc.tensor.matmul(ypsA[:, q0:QCH],
                                         lhsT=v_sb[:, j, hA, :],
                                         rhs=pt[:, 0, q0:QCH], start=st, stop=sp,
                                         skip_group_check=True)
                        nc.tensor.matmul(ypsB[:, q0:QCH],
                                         lhsT=v_sb[:, j, hB, :],
                                         rhs=pt[:, 1, q0:QCH], start=st, stop=sp,
                                         skip_group_check=True)

                    pend_av = None
                    for j in range(kmax):
                        jr = j - cqi * (QCH // 128)
                        q0 = max(jr, 0) * 128
                        sq_sl = slice(cqi * QCH + q0, (cqi + 1) * QCH)
                        s2 = ps_s2.tile([128, 2, QCH], F32, tag="s2")
                        nc.tensor.matmul(s2[:, 0, q0:QCH],
                                         lhsT=kT[0:64, j * 128:(j + 1) * 128],
                                         rhs=qT[0:64, sq_sl], start=True, stop=True,
                                         skip_group_check=True)
                        nc.tensor.matmul(s2[:, 1, q0:QCH],
                                         lhsT=kT[64:128, j * 128:(j + 1) * 128],
                                         rhs=qT[64:128, sq_sl], start=True, stop=True,
                                         skip_group_check=True)
                        pt = ptp.tile([128, 2, QCH], BF16, tag="pt")
                        nc.scalar.activation(pt[:, :, q0:QCH], s2[:, :, q0:QCH],
                                             mybir.ActivationFunctionType.Exp,
                                             scale=0.125)
                        if jr >= 0:
                            ptc = pt[:, :, q0:q0 + 128]
                            nc.vector.tensor_mul(ptc, ptc, _fbcast2(mask_sb[:]))
                        # AV(j) is issued after QK(j+1) so the in-order PE
                        # never parks on exp(j); exp runs under QK(j+1).
                        if pend_av is not None:
                            issue_av(*pend_av)
                        pend_av = (pt, j, q0)
                    issue_av(*pend_av)
                    sl = slice(cqi * QCH, (cqi + 1) * QCH)
                    drA = evw.tile([64, QCH], F32, tag="ev")
                    nc.vector.reciprocal_approx_fast(drA[:], ypsA[64:128, :])
                    drB = evw.tile([64, QCH], F32, tag="ev")
                    nc.vector.reciprocal_approx_fast(drB[:], ypsB[64:128, :])
                    # (GPSIMD cannot read PSUM on real hw - keep these on DVE)
                    nc.vector.tensor_mul(aoT_sb[0:64, p, sl], ypsA[0:64, :], drA[:])
                    nc.vector.tensor_mul(aoT_sb[64:128, p, sl], ypsB[0:64, :], drB[:])

            # ---- v projection first (needed by every attention pair) ----
            for tt in range(NTT):
                pss = ps_b.tile([128, FL], F32, tag="mm")
                for k in range(CT):
                    nc.tensor.matmul(
                        pss,
                        lhsT=xT_sb[:, k, tt * 128:(tt + 1) * 128],
                        rhs=wv_sb[:, k, :],
                        start=(k == 0),
                        stop=(k == CT - 1),
                    )
                nc.scalar.copy(v_sb[:, tt, :, 0:64], pss)

            # ---- per-pair: qkv -> rope -> attention -> exchange ----
            # qkv+rope of pair p+1 are issued before attention(p) so rope's
            # DVE work overlaps attention's PE work instead of stalling it.
            qkv_mtile(0)
            qkv_mtile(PAIRS)
            rope_mtile(0)
            rope_mtile(PAIRS)
            for p in range(PAIRS):
                if p + 1 < PAIRS:
                    qkv_mtile(p + 1)
                    qkv_mtile(PAIRS + p + 1)
                    rope_mtile(p + 1)
                    rope_mtile(PAIRS + p + 1)
                attention_pair(p)
                nc.sync.dma_start(cc_ins[p][:], aoT_sb[:, p, :])
                if not no_cc:
                    nc.gpsimd.collective_compute(
                        "AllGather",
                        mybir.AluOpType.bypass,
                        replica_groups=[[0, 1], [2, 3], [4, 5], [6, 7]],
                        ins=[cc_ins[p][:].opt()],
                        outs=[cc_outs[p][:].opt()],
                    )
                # qk slots p and 4+p are dead after attention p: receive the
                # gathered pair there (slot index == global f-tile index)
                if no_cc:
                    nc.sync.dma_start(qk_sb[:, p, :], cc_ins[p][:])
                    nc.sync.dma_start(qk_sb[:, PAIRS + p, :], cc_ins[p][:])
                else:
                    nc.sync.dma_start(qk_sb[:, p, :], cc_outs[p][0])
                    nc.sync.dma_start(qk_sb[:, PAIRS + p, :], cc_outs[p][1])

            early_ctx.close()

            # ---- projection over all 8 global f-tiles (rank-independent) ----
            for tt in range(NTT):
                pss = ps_b.tile([128, FL], F32, tag="mm")
                # kf in exchange-arrival order (pair p delivers kf=p and kf=4+p)
                kf_order = [kf for p in range(PAIRS) for kf in (p, PAIRS + p)]
                for i, kf in enumerate(kf_order):
                    nc.tensor.matmul(
                        pss,
                        lhsT=qk_sb[:, kf, tt * 128:(tt + 1) * 128],
                        rhs=wp_sb[:, kf, :],
                        start=(i == 0),
                        stop=(i == 2 * PAIRS - 1),
                    )
                ysb = evw.tile([128, FL], F32, tag="ev")
                nc.scalar.copy(ysb[:], pss)
                nc.sync.dma_start(y_d[tt * 128:(tt + 1) * 128, :], ysb[:])

    nc.compile()
    return nc


def _prep_core_inputs(x, Wqkv, Wproj, q_norm_w, k_norm_w, core):
    b, g = core // 2, core % 2
    bf = ml_dtypes.bfloat16
    xT = np.ascontiguousarray(x[b].T).astype(bf)
    cols = slice(g * FL, (g + 1) * FL)
    wq = Wqkv[:, 0:C][:, cols] * np.tile(q_norm_w, H_LOCAL)[None, :]
    wk = Wqkv[:, C:2 * C][:, cols] * np.tile(k_norm_w, H_LOCAL)[None, :]
    wv = Wqkv[:, 2 * C:3 * C][:, cols]
    wp = Wproj[:, cols]
    return {
        "xT": xT,
        "Wq": np.ascontiguousarray(wq).astype(bf),
        "Wk": np.ascontiguousarray(wk).astype(bf),
        "Wv": np.ascontiguousarray(wv).astype(bf),
        "Wp": np.ascontiguousarray(wp).astype(bf),
    }


def kernel(x, Wqkv, Wproj, q_norm_w, k_norm_w):
    if "nc" not in _cached:
        _cached["nc"] = build_program()
    nc = _cached["nc"]

    x = np.asarray(x, dtype=np.float32)
    Wqkv = np.asarray(Wqkv, dtype=np.float32)
    Wproj = np.asarray(Wproj, dtype=np.float32)
    q_norm_w = np.asarray(q_norm_w, dtype=np.float32)
    k_norm_w = np.asarray(k_norm_w, dtype=np.float32)

    in_maps = [
        _prep_core_inputs(x, Wqkv, Wproj, q_norm_w, k_norm_w, c) for c in range(8)
    ]
    res = run_bass_kernel_spmd(nc, in_maps, list(range(8)))
    outs = res.results

    y = np.empty((B, T, C), dtype=np.float32)
    for b in range(B):
        y[b, :, 0:FL] = outs[2 * b]["y"]
        y[b, :, FL:C] = outs[2 * b + 1]["y"]
    return y
